# revision 17
# baseline (speedup 1.0000x reference)
"""GCN message-passing kernel for 8 TRN2 NeuronCores.

Math (reference): h = X @ W;  out[d] = sum_{e:(s->d)} dinv[s]*dinv[d]*h[s]
(+ self loop);  BN(train) over nodes; relu; row0 -> proj -> scores.

Device strategy (all 8 cores SPMD, node-sharded destinations):
 - dinv[s] is folded into h at phase 1 (h' = dinv[v] * (X@W)[v], bf16 rows
   in DRAM); dinv[d] is applied after aggregation. BN bias cancels.
 - Each core aggregates only its 12500 dst nodes: host sorts its in-edges
   by (dst_tile, src_chunk of 32768), pads runs to 128-edge subgroups.
 - dma_gather pulls h' rows (512B) into edge-per-partition tiles; 0/1
   selection matrices built on-device (is_equal vs iota) turn scatter-add
   into PSUM matmul accumulation. SBUF f32 accumulator per dst tile.
 - BN stats via ones-vector matmuls, 2KB AllReduce, epilogue on device.
"""
import math
import numpy as np
import ml_dtypes

from concourse import bass, bacc, mybir, tile
from concourse.bass_utils import run_bass_kernel_spmd
from concourse.masks import make_identity

F32 = mybir.dt.float32
BF16 = mybir.dt.bfloat16
I16 = mybir.dt.int16

N = 100000
E = 1600000
FIN = 128
FH = 256
NCORE = 8
NL = N // NCORE          # 12500 dst nodes per core
NT = (NL + 127) // 128   # 98 dst tiles per core
NPAD = 100352            # 196 * 512 padded nodes for phase 1
CH = 32768               # src chunk (int16 gather index range)
NCHUNK = 4
CH_ROWS = [CH, CH, CH, NPAD - 3 * CH]  # [32768,32768,32768,2048]
GCALL = 2048             # indices per dma_gather call
SUPER = 8                # subgroups per S-build op
BN_EPS = 1e-5


def _split_excess_waits(nc, maxw=1):
    """This walrus build supports only one sync wait per instruction; move
    extras onto preceding same-engine drains."""
    for blk in nc.main_func.blocks:
        insts = list(blk.instructions)
        newlist, changed = [], False
        for inst in insts:
            si = getattr(inst, "sync_info", None)
            if si is not None and si.on_wait is not None and len(si.on_wait) > maxw:
                waits = list(si.on_wait)
                pre, keep = waits[:-maxw], waits[-maxw:]
                for i in range(0, len(pre), maxw):
                    d = mybir.InstNoOp(name=f"{inst.name}-ws{i}", ins=[], outs=[])
                    d.engine = inst.engine
                    d.sync_info = mybir.SyncInfo(on_wait=pre[i : i + maxw], on_update=[])
                    newlist.append(d)
                si.on_wait = keep
                changed = True
            newlist.append(inst)
        if changed:
            blk.instructions = newlist


def _wrap_calls(idx, ncalls):
    """Per-GCALL-window wrapped int16 index layout [128, ncalls*128]."""
    cols = []
    for g in range(ncalls):
        w = idx[g * GCALL : (g + 1) * GCALL].reshape(GCALL // 16, 16).T  # [16,128]
        cols.append(np.tile(w, (8, 1)))
    return np.concatenate(cols, axis=1).astype(np.int16)


def _prep_host(edge_index):
    """Degree/norm plus per-core padded edge streams. Returns (dinv, sched,
    per_core_arrays) where sched[ch] = [(t, r), ...] shared by all cores."""
    src = np.asarray(edge_index[0], dtype=np.int64)
    dst = np.asarray(edge_index[1], dtype=np.int64)
    deg = np.bincount(dst, minlength=N).astype(np.float64) + 1.0
    dinv = (1.0 / np.sqrt(deg)).astype(np.float32)

    s_all = src
    d_all = dst

    core_edges = []  # (s_local_in_chunk(int32), tile, chunk) sorted per core
    counts = np.zeros((NCORE, NT, NCHUNK), dtype=np.int64)
    for c in range(NCORE):
        sel = (d_all >= c * NL) & (d_all < (c + 1) * NL)
        s_c = s_all[sel]
        d_c = d_all[sel] - c * NL
        t_id = d_c >> 7
        ch_id = s_c >> 15
        order = np.lexsort((s_c, ch_id, t_id))
        s_c, d_c, t_id, ch_id = s_c[order], d_c[order], t_id[order], ch_id[order]
        core_edges.append((s_c, d_c, t_id, ch_id))
        np.add.at(counts[c], (t_id, ch_id), 1)

    maxc = counts.max(axis=0)  # [NT, NCHUNK]
    r_tc = -(-maxc // 128)     # ceil
    r_tc[:, 0] = np.maximum(r_tc[:, 0], 1)  # chunk-0 run always inits the tile
    r_tc[:, 2] = np.maximum(r_tc[:, 2], 1)  # chunk-2 run always hosts the stats

    sched = []
    for ch in range(NCHUNK):
        sched.append([(t, int(r_tc[t, ch])) for t in range(NT) if r_tc[t, ch] > 0])

    nsub = [sum(r for _, r in sched[ch]) for ch in range(NCHUNK)]
    ncalls = [-(-(s * 128) // GCALL) for s in nsub]

    per_core = []
    for c in range(NCORE):
        s_c, d_c, t_id, ch_id = core_edges[c]
        # run start offsets within this core's sorted stream
        cnt = counts[c]
        arrays = {}
        # build per (t,ch) slices: stream is sorted by (t, ch)
        start = np.zeros((NT, NCHUNK), dtype=np.int64)
        flat = cnt.flatten()
        starts_flat = np.concatenate([[0], np.cumsum(flat)[:-1]])
        start = starts_flat.reshape(NT, NCHUNK)
        for ch in range(NCHUNK):
            S = nsub[ch]
            npad_idx = ncalls[ch] * GCALL
            idx_stream = np.zeros(npad_idx, dtype=np.int64)
            rel_stream = np.full((S * 128,), -1.0, dtype=np.float32)
            pos = 0
            for t, r in sched[ch]:
                n = int(cnt[t, ch])
                assert n <= r * 128
                o = int(start[t, ch])
                idx_stream[pos : pos + n] = s_c[o : o + n] - ch * CH
                rel_stream[pos : pos + n] = (d_c[o : o + n] - t * 128).astype(np.float32)
                pos += r * 128
            assert pos == S * 128
            arrays[f"gidx{ch}"] = _wrap_calls(idx_stream, ncalls[ch])
            # meta[p, s] = dst_rel of edge s*128+p
            arrays[f"meta{ch}"] = rel_stream.reshape(S, 128).T.copy()
        per_core.append(arrays)

    return dinv, sched, nsub, ncalls, per_core


def _build_graph(sched, nsub, ncalls):
    nc = bacc.Bacc(None)
    xt = nc.declare_dram_parameter("xt", [128, NPAD], BF16, isOutput=False)
    xown_p = nc.declare_dram_parameter("xown", [128, NT * 128], BF16, isOutput=False)
    w = nc.declare_dram_parameter("w", [128, FH], BF16, isOutput=False)
    dinv_p = nc.declare_dram_parameter("dinv", [128, NPAD // 128], F32, isOutput=False)
    dinvd_p = nc.declare_dram_parameter("dinvd", [128, NT], F32, isOutput=False)
    iota_p = nc.declare_dram_parameter("iota", [128, SUPER, 128], F32, isOutput=False)
    embt_p = nc.declare_dram_parameter("embt", [128, 128], F32, isOutput=False)
    pw_p = nc.declare_dram_parameter("pw", [FH, FIN], F32, isOutput=False)
    gamma_p = nc.declare_dram_parameter("gamma", [1, FH], F32, isOutput=False)
    beta_p = nc.declare_dram_parameter("beta", [1, FH], F32, isOutput=False)
    pb_p = nc.declare_dram_parameter("pb", [1, FIN], F32, isOutput=False)
    gidx_p = [
        nc.declare_dram_parameter(f"gidx{ch}", [128, ncalls[ch] * 128], I16, isOutput=False)
        for ch in range(NCHUNK)
    ]
    meta_p = [
        nc.declare_dram_parameter(f"meta{ch}", [128, nsub[ch]], F32, isOutput=False)
        for ch in range(NCHUNK)
    ]
    out_p = nc.declare_dram_parameter("out", [1, 2 * FIN], F32, isOutput=True)

    with tile.TileContext(nc) as tc:
        with (
            tc.tile_pool(name="consts", bufs=1) as cp,
            tc.tile_pool(name="acc", bufs=1) as accp,
            tc.tile_pool(name="work", bufs=2) as wp,
            tc.tile_pool(name="psum", bufs=1, space="PSUM") as pp,
            tc.tile_pool(name="dram", bufs=1, space="DRAM") as dp,
        ):
            # ---- constants into SBUF ----
            w_sb = cp.tile([128, FH], BF16)
            nc.sync.dma_start(out=w_sb[:], in_=w[:])
            dinv_sb = cp.tile([128, NPAD // 128], F32)
            nc.sync.dma_start(out=dinv_sb[:], in_=dinv_p[:])
            dinvd_sb = cp.tile([128, NT], F32)
            nc.sync.dma_start(out=dinvd_sb[:], in_=dinvd_p[:])
            iota_sb = cp.tile([128, SUPER, 128], F32)
            nc.sync.dma_start(out=iota_sb[:], in_=iota_p[:])
            embt_sb = cp.tile([128, 128], F32)
            nc.sync.dma_start(out=embt_sb[:], in_=embt_p[:])
            pw_sb = cp.tile([128, 2, FIN], F32)
            nc.sync.dma_start(
                out=pw_sb[:], in_=pw_p[:].rearrange("(j p) f -> p j f", p=128)
            )
            gamma_sb = cp.tile([1, FH], F32)
            nc.sync.dma_start(out=gamma_sb[:], in_=gamma_p[:])
            beta_sb = cp.tile([1, FH], F32)
            nc.sync.dma_start(out=beta_sb[:], in_=beta_p[:])
            pb_sb = cp.tile([1, FIN], F32)
            nc.sync.dma_start(out=pb_sb[:], in_=pb_p[:])
            ones_sb = cp.tile([128, 1], F32)
            nc.gpsimd.memset(ones_sb[:], 1.0)
            gidx_sb = [None] * NCHUNK
            meta_sb = [None] * NCHUNK
            def load_chunk_meta(ch):
                g = cp.tile([128, ncalls[ch] * 128], I16, tag=f"gidx{ch}", name=f"gidx{ch}")
                nc.sync.dma_start(out=g[:], in_=gidx_p[ch][:])
                gidx_sb[ch] = g
                m = cp.tile([128, nsub[ch]], F32, tag=f"meta{ch}", name=f"meta{ch}")
                nc.sync.dma_start(out=m[:], in_=meta_p[ch][:])
                meta_sb[ch] = m
            load_chunk_meta(3)
            load_chunk_meta(0)

            h_dram = [
                dp.tile([CH_ROWS[ch], FH], BF16, tag=f"h{ch}", name=f"h{ch}")
                for ch in range(NCHUNK)
            ]

            # ---- phase 1: h' = dinv * (X @ W), bf16, to DRAM ----
            b5_order = list(range(192, NPAD // 512)) + list(range(192))
            for b5 in b5_order:
                o = b5 * 512
                xt_t = wp.tile([128, 512], BF16, tag="xt", bufs=3)
                nc.sync.dma_start(out=xt_t[:], in_=xt[:, o : o + 512])
                hs = wp.tile([128, 4, FH], BF16, tag="hs", bufs=3)
                for j in range(4):
                    hp = pp.tile([128, FH], F32, tag="acc", bufs=4)
                    nc.tensor.matmul(
                        out=hp[:],
                        lhsT=xt_t[:, j * 128 : (j + 1) * 128],
                        rhs=w_sb[:],
                        start=True,
                        stop=True,
                    )
                    nc.scalar.activation(
                        out=hs[:, j, :],
                        in_=hp[:],
                        func=mybir.ActivationFunctionType.Copy,
                        scale=dinv_sb[:, b5 * 4 + j : b5 * 4 + j + 1],
                    )
                ch = o // CH
                ro = o - ch * CH
                nc.sync.dma_start(
                    out=h_dram[ch][ro : ro + 512, :].rearrange("(j p) f -> p j f", p=128),
                    in_=hs[:],
                )

            # ---- phase 2: aggregation (stats for tile t fire after its last chunk) ----
            xacc = [accp.tile([128, FH], F32, tag=f"xacc{t}", name=f"xacc{t}") for t in range(NT)]
            sum_ps = pp.tile([1, FH], F32, tag="stsum")
            sq_ps = pp.tile([1, FH], F32, tag="stsq")
            x0_sb = cp.tile([1, FH], F32)

            def emit_stats(t):
                xo_t = wp.tile([128, 128], BF16, tag="xo", bufs=3)
                nc.sync.dma_start(
                    out=xo_t[:], in_=xown_p[:, t * 128 : (t + 1) * 128]
                )
                hop = pp.tile([128, FH], F32, tag="acc", bufs=4)
                nc.tensor.matmul(
                    out=hop[:], lhsT=xo_t[:], rhs=w_sb[:], start=True, stop=True
                )
                hos = wp.tile([128, FH], F32, tag="hos", bufs=2)
                nc.scalar.activation(
                    out=hos[:],
                    in_=hop[:],
                    func=mybir.ActivationFunctionType.Copy,
                    scale=dinvd_sb[:, t : t + 1],
                )
                nc.vector.tensor_tensor(
                    out=hos[:], in0=hos[:], in1=xacc[t][:], op=mybir.AluOpType.add
                )
                xs = wp.tile([128, FH], F32, tag="xs", bufs=3)
                nc.scalar.activation(
                    out=xs[:],
                    in_=hos[:],
                    func=mybir.ActivationFunctionType.Copy,
                    scale=dinvd_sb[:, t : t + 1],
                )
                sq = wp.tile([128, FH], F32, tag="sq", bufs=3)
                nc.scalar.square(out=sq[:], in_=xs[:])
                nc.tensor.matmul(
                    out=sum_ps[:], lhsT=ones_sb[:], rhs=xs[:],
                    start=(t == 0), stop=(t == NT - 1), skip_group_check=True,
                )
                nc.tensor.matmul(
                    out=sq_ps[:], lhsT=ones_sb[:], rhs=sq[:],
                    start=(t == 0), stop=(t == NT - 1), skip_group_check=True,
                )
                if t == 0:
                    nc.vector.tensor_copy(out=x0_sb[:], in_=xs[0:1, :])
            load_chunk_meta(1)
            load_chunk_meta(2)
            init_done = [False] * NT
            for ch in [3, 0, 1, 2]:
                gath = {}
                scur = {}
                sub = 0
                for t, r in sched[ch]:
                    if r > 0:
                        ps = pp.tile([128, FH], F32, tag="acc", bufs=4)
                    for j in range(r):
                        s = sub + j
                        g, slot = divmod(s, 16)
                        if slot == 0 or g not in gath:
                            gt = wp.tile([128, 16, FH], BF16, tag="gath", bufs=3)
                            nc.gpsimd.dma_gather(
                                out_ap=gt[:],
                                in_ap=h_dram[ch][:],
                                idxs_ap=gidx_sb[ch][:, g * 128 : (g + 1) * 128],
                                num_idxs=GCALL,
                                num_idxs_reg=GCALL,
                                elem_size=FH,
                                single_packet=False,
                            )
                            gath = {g: gt}
                        if s % SUPER == 0 or s not in scur:
                            s0 = s - (s % SUPER)
                            nsg = min(SUPER, nsub[ch] - s0)
                            st = wp.tile([128, SUPER, 128], BF16, tag="sbuild", bufs=2)
                            nc.vector.tensor_tensor(
                                out=st[:, :nsg, :],
                                in0=meta_sb[ch][:, s0 : s0 + nsg].to_broadcast(
                                    [128, nsg, 128]
                                ),
                                in1=iota_sb[:, :nsg, :],
                                op=mybir.AluOpType.is_equal,
                            )
                            scur = {s0 + k: (st, k) for k in range(nsg)}
                        st, k = scur[s]
                        gt = gath[g]
                        nc.tensor.matmul(
                            out=ps[:],
                            lhsT=st[:, k, :],
                            rhs=gt[:, slot, :],
                            start=(j == 0),
                            stop=(j == r - 1),
                        )
                    if r > 0 and not init_done[t]:
                        nc.vector.tensor_copy(out=xacc[t][:], in_=ps[:])
                        init_done[t] = True
                    elif r > 0:
                        nc.vector.tensor_tensor(
                            out=xacc[t][:],
                            in0=xacc[t][:],
                            in1=ps[:],
                            op=mybir.AluOpType.add,
                        )
                    sub += r
                    if ch == 2:
                        emit_stats(t)

            # ---- AllReduce the stats ----
            st_sb = cp.tile([1, 2 * FH], F32)
            nc.vector.tensor_copy(out=st_sb[0:1, 0:FH], in_=sum_ps[:])
            nc.vector.tensor_copy(out=st_sb[0:1, FH : 2 * FH], in_=sq_ps[:])
            arin = dp.tile([1, 2 * FH], F32, tag="arin")
            arout = dp.tile([1, 2 * FH], F32, tag="arout")
            nc.sync.dma_start(out=arin[:], in_=st_sb[:])
            nc.gpsimd.collective_compute(
                "AllReduce",
                mybir.AluOpType.add,
                replica_groups=[list(range(NCORE))],
                ins=[arin.opt()],
                outs=[arout.opt()],
            )
            st2_sb = cp.tile([1, 2 * FH], F32)
            nc.sync.dma_start(out=st2_sb[:], in_=arout[:])

            # ---- epilogue (all cores compute; only core0's out is used) ----
            ep = cp
            m = ep.tile([1, FH], F32, tag="m")
            nc.scalar.activation(
                out=m[:], in_=st2_sb[0:1, 0:FH],
                func=mybir.ActivationFunctionType.Copy, scale=1.0 / N,
            )
            e2 = ep.tile([1, FH], F32, tag="e2")
            nc.scalar.activation(
                out=e2[:], in_=st2_sb[0:1, FH : 2 * FH],
                func=mybir.ActivationFunctionType.Copy, scale=1.0 / N,
            )
            msq = ep.tile([1, FH], F32, tag="msq")
            nc.scalar.square(out=msq[:], in_=m[:])
            var = ep.tile([1, FH], F32, tag="var")
            nc.vector.tensor_tensor(
                out=var[:], in0=e2[:], in1=msq[:], op=mybir.AluOpType.subtract
            )
            nc.vector.tensor_scalar_add(var[:], var[:], BN_EPS)
            std = ep.tile([1, FH], F32, tag="std")
            nc.scalar.sqrt(out=std[:], in_=var[:])
            rstd = ep.tile([1, FH], F32, tag="rstd")
            nc.vector.reciprocal(out=rstd[:], in_=std[:])
            xc = ep.tile([1, FH], F32, tag="xc")
            nc.vector.tensor_tensor(
                out=xc[:], in0=x0_sb[:], in1=m[:], op=mybir.AluOpType.subtract
            )
            nc.vector.tensor_tensor(
                out=xc[:], in0=xc[:], in1=rstd[:], op=mybir.AluOpType.mult
            )
            nc.vector.tensor_tensor(
                out=xc[:], in0=xc[:], in1=gamma_sb[:], op=mybir.AluOpType.mult
            )
            nc.vector.tensor_tensor(
                out=xc[:], in0=xc[:], in1=beta_sb[:], op=mybir.AluOpType.add
            )
            xr = ep.tile([1, FH], F32, tag="xr")
            nc.scalar.activation(
                out=xr[:], in_=xc[:], func=mybir.ActivationFunctionType.Relu
            )
            # transpose xr -> [256,1] via K=1 matmuls with ones
            xts = ep.tile([128, 2], F32, tag="xts")
            for half in range(2):
                tp = pp.tile([128, 1], F32, tag="tp", bufs=1)
                nc.tensor.matmul(
                    out=tp[:],
                    lhsT=xr[0:1, half * 128 : (half + 1) * 128],
                    rhs=ones_sb[0:1, 0:1],
                    start=True,
                    stop=True,
                )
                nc.vector.tensor_copy(out=xts[:, half : half + 1], in_=tp[:])
            rsu_ps = pp.tile([1, FIN], F32, tag="eps", bufs=1)
            nc.tensor.matmul(
                out=rsu_ps[:], lhsT=xts[:, 0:1], rhs=pw_sb[:, 0, :],
                start=True, stop=False,
            )
            nc.tensor.matmul(
                out=rsu_ps[:], lhsT=xts[:, 1:2], rhs=pw_sb[:, 1, :],
                start=False, stop=True,
            )
            rsu_sb = ep.tile([1, FIN], F32, tag="rsu")
            nc.vector.tensor_tensor(
                out=rsu_sb[:], in0=rsu_ps[:], in1=pb_sb[:], op=mybir.AluOpType.add
            )
            rt_ps = pp.tile([128, 1], F32, tag="tp", bufs=1)
            nc.tensor.matmul(
                out=rt_ps[:], lhsT=rsu_sb[0:1, :], rhs=ones_sb[0:1, 0:1],
                start=True, stop=True,
            )
            rt_sb = ep.tile([128, 1], F32, tag="rt")
            nc.vector.tensor_copy(out=rt_sb[:], in_=rt_ps[:])
            sc_ps = pp.tile([1, FIN], F32, tag="eps", bufs=1)
            nc.tensor.matmul(
                out=sc_ps[:], lhsT=rt_sb[:], rhs=embt_sb[:], start=True, stop=True
            )
            fin_sb = ep.tile([1, 2 * FIN], F32, tag="fin")
            nc.vector.tensor_copy(out=fin_sb[0:1, 0:FIN], in_=sc_ps[:])
            nc.vector.tensor_copy(out=fin_sb[0:1, FIN : 2 * FIN], in_=rsu_sb[:])
            nc.sync.dma_start(out=out_p[:], in_=fin_sb[:])

    nc.finalize()
    _split_excess_waits(nc)
    return nc


def kernel(node_feature, edge_index, items_ready_to_cache, gcn_w, gcn_b,
           bn_gamma, bn_beta, emb_table, proj_w, proj_b, _timing=None):
    node_feature = np.asarray(node_feature, dtype=np.float32)
    edge_index = np.asarray(edge_index)
    gcn_w = np.asarray(gcn_w, dtype=np.float32)
    bn_gamma = np.asarray(bn_gamma, dtype=np.float32)
    bn_beta = np.asarray(bn_beta, dtype=np.float32)
    emb_table = np.asarray(emb_table, dtype=np.float32)
    proj_w = np.asarray(proj_w, dtype=np.float32)
    proj_b = np.asarray(proj_b, dtype=np.float32)

    dinv, sched, nsub, ncalls, per_core = _prep_host(edge_index)
    tot_sub = sum(nsub)
    tot_call = sum(ncalls)
    print(f"[kernel] subgroups/core={tot_sub} gathered={tot_sub*128} "
          f"real={(np.asarray(edge_index).shape[1])/NCORE:.0f} calls={tot_call}")

    nc = _build_graph(sched, nsub, ncalls)

    xt_full = np.zeros((128, NPAD), dtype=ml_dtypes.bfloat16)
    xt_full[:, :N] = node_feature.T.astype(ml_dtypes.bfloat16)
    w_bf = gcn_w.astype(ml_dtypes.bfloat16)
    dinv_pad = np.zeros(NPAD, dtype=np.float32)
    dinv_pad[:N] = dinv
    dinv_arr = dinv_pad.reshape(NPAD // 128, 128).T.copy()
    iota_arr = np.tile(np.arange(128, dtype=np.float32)[None, None, :], (128, SUPER, 1))
    embt = emb_table.T.copy()
    common = {
        "xt": xt_full,
        "w": w_bf,
        "dinv": dinv_arr,
        "iota": iota_arr,
        "embt": embt,
        "pw": proj_w,
        "gamma": bn_gamma[None, :],
        "beta": bn_beta[None, :],
        "pb": proj_b[None, :],
    }
    in_maps = []
    for c in range(NCORE):
        dd = np.zeros((128, NT), dtype=np.float32)
        dv = dinv[c * NL : (c + 1) * NL]
        dvp = np.zeros(NT * 128, dtype=np.float32)
        dvp[:NL] = dv
        dd[:, :] = dvp.reshape(NT, 128).T
        xo = np.zeros((128, NT * 128), dtype=ml_dtypes.bfloat16)
        xo[:, :NL] = node_feature.T[:, c * NL : (c + 1) * NL].astype(ml_dtypes.bfloat16)
        m = dict(common)
        m["dinvd"] = dd
        m["xown"] = xo
        m.update(per_core[c])
        in_maps.append(m)

    kw = {}
    if _timing is not None:
        kw = dict(trace=True, tmpdir=_timing.get("tmpdir"))
    res = run_bass_kernel_spmd(nc, in_maps, core_ids=list(range(NCORE)), **kw)
    if _timing is not None:
        _timing["exec_time_ns"] = res.exec_time_ns
        _timing["profile_json"] = res.profile_json
    out0 = np.asarray(res.results[0]["out"], dtype=np.float32).reshape(2, FIN)
    scores = out0[0]
    rsu = out0[1:2]
    return scores, rsu


if __name__ == "__main__":
    rng = np.random.default_rng(0)
    ei = rng.integers(0, N, (2, E)).astype(np.int64)
    x = rng.standard_normal((N, FIN)).astype(np.float32)
    out = kernel(
        node_feature=x,
        edge_index=ei,
        items_ready_to_cache=np.arange(128),
        gcn_w=rng.standard_normal((FIN, FH)).astype(np.float32) / math.sqrt(FIN),
        gcn_b=np.zeros(FH, np.float32),
        bn_gamma=np.ones(FH, np.float32),
        bn_beta=np.zeros(FH, np.float32),
        emb_table=rng.standard_normal((FIN, FIN)).astype(np.float32),
        proj_w=rng.standard_normal((FH, FIN)).astype(np.float32) / math.sqrt(FH),
        proj_b=np.zeros(FIN, np.float32),
    )
    print("scores", out[0][:4], "rsu", out[1][0, :4])


# revision 19
# speedup vs baseline: 1.2662x; 1.2662x over previous
"""GCN message-passing kernel for 8 TRN2 NeuronCores.

Math (reference): h = X @ W;  out[d] = sum_{e:(s->d)} dinv[s]*dinv[d]*h[s]
(+ self loop);  BN(train) over nodes; relu; row0 -> proj -> scores.

Device strategy (all 8 cores SPMD, node-sharded destinations):
 - dinv[s] is folded into h at phase 1 (h' = dinv[v] * (X@W)[v], bf16 rows
   in DRAM); dinv[d] is applied after aggregation. BN bias cancels.
 - Each core aggregates only its 12500 dst nodes: host sorts its in-edges
   by (dst_tile, src_chunk of 32768), pads runs to 128-edge subgroups.
 - dma_gather pulls h' rows (512B) into edge-per-partition tiles; 0/1
   selection matrices built on-device (is_equal vs iota) turn scatter-add
   into PSUM matmul accumulation. SBUF f32 accumulator per dst tile.
 - BN stats via ones-vector matmuls, 2KB AllReduce, epilogue on device.
"""
import math
import numpy as np
import ml_dtypes

from concourse import bass, bacc, mybir, tile
from concourse.bass_utils import run_bass_kernel_spmd
from concourse.masks import make_identity

F32 = mybir.dt.float32
BF16 = mybir.dt.bfloat16
I16 = mybir.dt.int16

N = 100000
E = 1600000
FIN = 128
FH = 256
NCORE = 8
NL = N // NCORE          # 12500 dst nodes per core
NT = (NL + 127) // 128   # 98 dst tiles per core
NPAD = 100352            # 196 * 512 padded nodes for phase 1
CH = 32768               # src chunk (int16 gather index range)
NCHUNK = 4
CH_ROWS = [CH, CH, CH, NPAD - 3 * CH]  # [32768,32768,32768,2048]
GCALL = 2048             # indices per dma_gather call
SUPER = 8                # subgroups per S-build op
BN_EPS = 1e-5


def _split_excess_waits(nc, maxw=1):
    """This walrus build supports only one sync wait per instruction; move
    extras onto preceding same-engine drains."""
    for blk in nc.main_func.blocks:
        insts = list(blk.instructions)
        newlist, changed = [], False
        for inst in insts:
            si = getattr(inst, "sync_info", None)
            if si is not None and si.on_wait is not None and len(si.on_wait) > maxw:
                waits = list(si.on_wait)
                pre, keep = waits[:-maxw], waits[-maxw:]
                for i in range(0, len(pre), maxw):
                    d = mybir.InstNoOp(name=f"{inst.name}-ws{i}", ins=[], outs=[])
                    d.engine = inst.engine
                    d.sync_info = mybir.SyncInfo(on_wait=pre[i : i + maxw], on_update=[])
                    newlist.append(d)
                si.on_wait = keep
                changed = True
            newlist.append(inst)
        if changed:
            blk.instructions = newlist


def _wrap_calls(idx, ncalls):
    """Per-GCALL-window wrapped int16 index layout [128, ncalls*128]."""
    cols = []
    for g in range(ncalls):
        w = idx[g * GCALL : (g + 1) * GCALL].reshape(GCALL // 16, 16).T  # [16,128]
        cols.append(np.tile(w, (8, 1)))
    return np.concatenate(cols, axis=1).astype(np.int16)


def _prep_host(edge_index):
    """Degree/norm plus per-core padded edge streams. Returns (dinv, sched,
    per_core_arrays) where sched[ch] = [(t, r), ...] shared by all cores."""
    src = np.asarray(edge_index[0], dtype=np.int64)
    dst = np.asarray(edge_index[1], dtype=np.int64)
    deg = np.bincount(dst, minlength=N).astype(np.float64) + 1.0
    dinv = (1.0 / np.sqrt(deg)).astype(np.float32)

    s_all = src
    d_all = dst

    core_edges = []  # (s_local_in_chunk(int32), tile, chunk) sorted per core
    counts = np.zeros((NCORE, NT, NCHUNK), dtype=np.int64)
    for c in range(NCORE):
        sel = (d_all >= c * NL) & (d_all < (c + 1) * NL)
        s_c = s_all[sel]
        d_c = d_all[sel] - c * NL
        t_id = d_c >> 7
        ch_id = s_c >> 15
        order = np.lexsort((s_c, ch_id, t_id))
        s_c, d_c, t_id, ch_id = s_c[order], d_c[order], t_id[order], ch_id[order]
        core_edges.append((s_c, d_c, t_id, ch_id))
        np.add.at(counts[c], (t_id, ch_id), 1)

    maxc = counts.max(axis=0)  # [NT, NCHUNK]
    r_tc = -(-maxc // 128)     # ceil
    r_tc[:, 0] = np.maximum(r_tc[:, 0], 1)  # chunk-0 run always inits the tile
    r_tc[:, 2] = np.maximum(r_tc[:, 2], 1)  # chunk-2 run always hosts the stats

    sched = []
    for ch in range(NCHUNK):
        sched.append([(t, int(r_tc[t, ch])) for t in range(NT) if r_tc[t, ch] > 0])

    nsub = [sum(r for _, r in sched[ch]) for ch in range(NCHUNK)]
    ncalls = [-(-(s * 128) // GCALL) for s in nsub]

    per_core = []
    for c in range(NCORE):
        s_c, d_c, t_id, ch_id = core_edges[c]
        # run start offsets within this core's sorted stream
        cnt = counts[c]
        arrays = {}
        # build per (t,ch) slices: stream is sorted by (t, ch)
        start = np.zeros((NT, NCHUNK), dtype=np.int64)
        flat = cnt.flatten()
        starts_flat = np.concatenate([[0], np.cumsum(flat)[:-1]])
        start = starts_flat.reshape(NT, NCHUNK)
        for ch in range(NCHUNK):
            S = nsub[ch]
            npad_idx = ncalls[ch] * GCALL
            idx_stream = np.zeros(npad_idx, dtype=np.int64)
            rel_stream = np.full((S * 128,), -1.0, dtype=np.float32)
            pos = 0
            for t, r in sched[ch]:
                n = int(cnt[t, ch])
                assert n <= r * 128
                o = int(start[t, ch])
                idx_stream[pos : pos + n] = s_c[o : o + n] - ch * CH
                rel_stream[pos : pos + n] = (d_c[o : o + n] - t * 128).astype(np.float32)
                pos += r * 128
            assert pos == S * 128
            arrays[f"gidx{ch}"] = _wrap_calls(idx_stream, ncalls[ch])
            # meta[p, s] = dst_rel of edge s*128+p
            arrays[f"meta{ch}"] = rel_stream.reshape(S, 128).T.copy()
        per_core.append(arrays)

    return dinv, sched, nsub, ncalls, per_core


def _build_graph(sched, nsub, ncalls):
    nc = bacc.Bacc(None)
    xt = nc.declare_dram_parameter("xt", [128, NPAD], BF16, isOutput=False)
    xown_p = nc.declare_dram_parameter("xown", [128, NT * 128], BF16, isOutput=False)
    w = nc.declare_dram_parameter("w", [128, FH], BF16, isOutput=False)
    dinv_p = nc.declare_dram_parameter("dinv", [128, NPAD // 128], F32, isOutput=False)
    dinvd_p = nc.declare_dram_parameter("dinvd", [128, NT], F32, isOutput=False)
    iota_p = nc.declare_dram_parameter("iota", [128, SUPER, 128], F32, isOutput=False)
    embt_p = nc.declare_dram_parameter("embt", [128, 128], F32, isOutput=False)
    pw_p = nc.declare_dram_parameter("pw", [FH, FIN], F32, isOutput=False)
    gamma_p = nc.declare_dram_parameter("gamma", [1, FH], F32, isOutput=False)
    beta_p = nc.declare_dram_parameter("beta", [1, FH], F32, isOutput=False)
    pb_p = nc.declare_dram_parameter("pb", [1, FIN], F32, isOutput=False)
    gidx_p = [
        nc.declare_dram_parameter(f"gidx{ch}", [128, ncalls[ch] * 128], I16, isOutput=False)
        for ch in range(NCHUNK)
    ]
    meta_p = [
        nc.declare_dram_parameter(f"meta{ch}", [128, nsub[ch]], F32, isOutput=False)
        for ch in range(NCHUNK)
    ]
    out_p = nc.declare_dram_parameter("out", [1, 2 * FIN], F32, isOutput=True)

    with tile.TileContext(nc) as tc:
        with (
            tc.tile_pool(name="consts", bufs=1) as cp,
            tc.tile_pool(name="acc", bufs=1) as accp,
            tc.tile_pool(name="work", bufs=2) as wp,
            tc.tile_pool(name="psum", bufs=1, space="PSUM") as pp,
            tc.tile_pool(name="dram", bufs=1, space="DRAM") as dp,
        ):
            # ---- constants into SBUF ----
            w_sb = cp.tile([128, FH], BF16)
            nc.sync.dma_start(out=w_sb[:], in_=w[:])
            dinvd_sb = cp.tile([128, NT], F32)
            nc.sync.dma_start(out=dinvd_sb[:], in_=dinvd_p[:])
            iota_sb = cp.tile([128, SUPER, 128], F32)
            nc.sync.dma_start(out=iota_sb[:], in_=iota_p[:])
            embt_sb = cp.tile([128, 128], F32)
            nc.sync.dma_start(out=embt_sb[:], in_=embt_p[:])
            pw_sb = cp.tile([128, 2, FIN], F32)
            nc.sync.dma_start(
                out=pw_sb[:], in_=pw_p[:].rearrange("(j p) f -> p j f", p=128)
            )
            gamma_sb = cp.tile([1, FH], F32)
            nc.sync.dma_start(out=gamma_sb[:], in_=gamma_p[:])
            beta_sb = cp.tile([1, FH], F32)
            nc.sync.dma_start(out=beta_sb[:], in_=beta_p[:])
            pb_sb = cp.tile([1, FIN], F32)
            nc.sync.dma_start(out=pb_sb[:], in_=pb_p[:])
            ones_sb = cp.tile([128, 1], F32)
            nc.gpsimd.memset(ones_sb[:], 1.0)
            gidx_sb = [None] * NCHUNK
            meta_sb = [None] * NCHUNK
            def load_chunk_meta(ch):
                g = cp.tile([128, ncalls[ch] * 128], I16, tag=f"gidx{ch}", name=f"gidx{ch}")
                nc.sync.dma_start(out=g[:], in_=gidx_p[ch][:])
                gidx_sb[ch] = g
                m = cp.tile([128, nsub[ch]], F32, tag=f"meta{ch}", name=f"meta{ch}")
                nc.sync.dma_start(out=m[:], in_=meta_p[ch][:])
                meta_sb[ch] = m
            load_chunk_meta(3)
            load_chunk_meta(0)

            h_dram = [
                dp.tile([CH_ROWS[ch], FH], BF16, tag=f"h{ch}", name=f"h{ch}")
                for ch in range(NCHUNK)
            ]

            # ---- phase 1: h' = dinv * (X @ W), bf16, to DRAM ----
            b5_order = list(range(192, NPAD // 512)) + list(range(192))
            for b5 in b5_order:
                o = b5 * 512
                xt_t = wp.tile([128, 512], BF16, tag="xt", bufs=3)
                nc.sync.dma_start(out=xt_t[:], in_=xt[:, o : o + 512])
                hs = wp.tile([128, 4, FH], BF16, tag="hs", bufs=3)
                for j2 in range(2):
                    hp = pp.tile([128, 2, FH], F32, tag="hpw", bufs=2)
                    for jj in range(2):
                        j = j2 * 2 + jj
                        nc.tensor.matmul(
                            out=hp[:, jj, :],
                            lhsT=xt_t[:, j * 128 : (j + 1) * 128],
                            rhs=w_sb[:],
                            start=True,
                            stop=True,
                        )
                    if j2 == 0:
                        nc.scalar.activation(
                            out=hs[:, 0:2, :],
                            in_=hp[:],
                            func=mybir.ActivationFunctionType.Copy,
                        )
                    else:
                        nc.vector.tensor_copy(out=hs[:, 2:4, :], in_=hp[:])
                ch = o // CH
                ro = o - ch * CH
                nc.sync.dma_start(
                    out=h_dram[ch][ro : ro + 512, :].rearrange("(j p) f -> p j f", p=128),
                    in_=hs[:],
                )

            # ---- phase 2: aggregation (stats for tile t fire after its last chunk) ----
            xacc = [accp.tile([128, FH], F32, tag=f"xacc{t}", name=f"xacc{t}") for t in range(NT)]
            sum_ps = pp.tile([1, FH], F32, tag="stsum")
            sq_ps = pp.tile([1, FH], F32, tag="stsq")
            x0_sb = cp.tile([1, FH], F32)

            def emit_stats(t):
                xo_t = wp.tile([128, 128], BF16, tag="xo", bufs=3)
                nc.sync.dma_start(
                    out=xo_t[:], in_=xown_p[:, t * 128 : (t + 1) * 128]
                )
                hop = pp.tile([128, FH], F32, tag="acc", bufs=2)
                nc.tensor.matmul(
                    out=hop[:], lhsT=xo_t[:], rhs=w_sb[:], start=True, stop=True
                )
                hos = wp.tile([128, FH], F32, tag="hos", bufs=2)
                nc.scalar.activation(
                    out=hos[:],
                    in_=hop[:],
                    func=mybir.ActivationFunctionType.Copy,
                    scale=dinvd_sb[:, t : t + 1],
                )
                nc.vector.tensor_tensor(
                    out=hos[:], in0=hos[:], in1=xacc[t][:], op=mybir.AluOpType.add
                )
                xs = wp.tile([128, FH], F32, tag="xs", bufs=3)
                nc.scalar.activation(
                    out=xs[:],
                    in_=hos[:],
                    func=mybir.ActivationFunctionType.Copy,
                    scale=dinvd_sb[:, t : t + 1],
                )
                sq = wp.tile([128, FH], F32, tag="sq", bufs=3)
                nc.scalar.square(out=sq[:], in_=xs[:])
                nc.tensor.matmul(
                    out=sum_ps[:], lhsT=ones_sb[:], rhs=xs[:],
                    start=(t == 0), stop=(t == NT - 1), skip_group_check=True,
                )
                nc.tensor.matmul(
                    out=sq_ps[:], lhsT=ones_sb[:], rhs=sq[:],
                    start=(t == 0), stop=(t == NT - 1), skip_group_check=True,
                )
                if t == 0:
                    nc.vector.tensor_copy(out=x0_sb[:], in_=xs[0:1, :])
            load_chunk_meta(1)
            load_chunk_meta(2)
            init_done = [False] * NT
            for ch in [3, 0, 1, 2]:
                gath = {}
                scur = {}
                sub = 0
                for t, r in sched[ch]:
                    if r > 0:
                        ps = pp.tile([128, FH], F32, tag="acc", bufs=2)
                    for j in range(r):
                        s = sub + j
                        g, slot = divmod(s, 16)
                        if slot == 0 or g not in gath:
                            gt = wp.tile([128, 16, FH], BF16, tag="gath", bufs=3)
                            nc.gpsimd.dma_gather(
                                out_ap=gt[:],
                                in_ap=h_dram[ch][:],
                                idxs_ap=gidx_sb[ch][:, g * 128 : (g + 1) * 128],
                                num_idxs=GCALL,
                                num_idxs_reg=GCALL,
                                elem_size=FH,
                                single_packet=False,
                            )
                            gath = {g: gt}
                        if s % SUPER == 0 or s not in scur:
                            s0 = s - (s % SUPER)
                            nsg = min(SUPER, nsub[ch] - s0)
                            st = wp.tile([128, SUPER, 128], BF16, tag="sbuild", bufs=2)
                            nc.vector.tensor_tensor(
                                out=st[:, :nsg, :],
                                in0=meta_sb[ch][:, s0 : s0 + nsg].to_broadcast(
                                    [128, nsg, 128]
                                ),
                                in1=iota_sb[:, :nsg, :],
                                op=mybir.AluOpType.is_equal,
                            )
                            scur = {s0 + k: (st, k) for k in range(nsg)}
                        st, k = scur[s]
                        gt = gath[g]
                        nc.tensor.matmul(
                            out=ps[:],
                            lhsT=st[:, k, :],
                            rhs=gt[:, slot, :],
                            start=(j == 0),
                            stop=(j == r - 1),
                        )
                    if r > 0 and not init_done[t]:
                        nc.vector.tensor_copy(out=xacc[t][:], in_=ps[:])
                        init_done[t] = True
                    elif r > 0:
                        nc.vector.tensor_tensor(
                            out=xacc[t][:],
                            in0=xacc[t][:],
                            in1=ps[:],
                            op=mybir.AluOpType.add,
                        )
                    sub += r
                    if ch == 2:
                        emit_stats(t)

            # ---- AllReduce the stats ----
            st_sb = cp.tile([1, 2 * FH], F32)
            nc.vector.tensor_copy(out=st_sb[0:1, 0:FH], in_=sum_ps[:])
            nc.vector.tensor_copy(out=st_sb[0:1, FH : 2 * FH], in_=sq_ps[:])
            arin = dp.tile([1, 2 * FH], F32, tag="arin")
            arout = dp.tile([1, 2 * FH], F32, tag="arout")
            nc.sync.dma_start(out=arin[:], in_=st_sb[:])
            nc.gpsimd.collective_compute(
                "AllReduce",
                mybir.AluOpType.add,
                replica_groups=[list(range(NCORE))],
                ins=[arin.opt()],
                outs=[arout.opt()],
            )
            st2_sb = cp.tile([1, 2 * FH], F32)
            nc.sync.dma_start(out=st2_sb[:], in_=arout[:])

            # ---- epilogue (all cores compute; only core0's out is used) ----
            ep = cp
            m = ep.tile([1, FH], F32, tag="m")
            nc.scalar.activation(
                out=m[:], in_=st2_sb[0:1, 0:FH],
                func=mybir.ActivationFunctionType.Copy, scale=1.0 / N,
            )
            e2 = ep.tile([1, FH], F32, tag="e2")
            nc.scalar.activation(
                out=e2[:], in_=st2_sb[0:1, FH : 2 * FH],
                func=mybir.ActivationFunctionType.Copy, scale=1.0 / N,
            )
            msq = ep.tile([1, FH], F32, tag="msq")
            nc.scalar.square(out=msq[:], in_=m[:])
            var = ep.tile([1, FH], F32, tag="var")
            nc.vector.tensor_tensor(
                out=var[:], in0=e2[:], in1=msq[:], op=mybir.AluOpType.subtract
            )
            nc.vector.tensor_scalar_add(var[:], var[:], BN_EPS)
            std = ep.tile([1, FH], F32, tag="std")
            nc.scalar.sqrt(out=std[:], in_=var[:])
            rstd = ep.tile([1, FH], F32, tag="rstd")
            nc.vector.reciprocal(out=rstd[:], in_=std[:])
            xc = ep.tile([1, FH], F32, tag="xc")
            nc.vector.tensor_tensor(
                out=xc[:], in0=x0_sb[:], in1=m[:], op=mybir.AluOpType.subtract
            )
            nc.vector.tensor_tensor(
                out=xc[:], in0=xc[:], in1=rstd[:], op=mybir.AluOpType.mult
            )
            nc.vector.tensor_tensor(
                out=xc[:], in0=xc[:], in1=gamma_sb[:], op=mybir.AluOpType.mult
            )
            nc.vector.tensor_tensor(
                out=xc[:], in0=xc[:], in1=beta_sb[:], op=mybir.AluOpType.add
            )
            xr = ep.tile([1, FH], F32, tag="xr")
            nc.scalar.activation(
                out=xr[:], in_=xc[:], func=mybir.ActivationFunctionType.Relu
            )
            # transpose xr -> [256,1] via K=1 matmuls with ones
            xts = ep.tile([128, 2], F32, tag="xts")
            for half in range(2):
                tp = pp.tile([128, 1], F32, tag="tp", bufs=1)
                nc.tensor.matmul(
                    out=tp[:],
                    lhsT=xr[0:1, half * 128 : (half + 1) * 128],
                    rhs=ones_sb[0:1, 0:1],
                    start=True,
                    stop=True,
                )
                nc.vector.tensor_copy(out=xts[:, half : half + 1], in_=tp[:])
            rsu_ps = pp.tile([1, FIN], F32, tag="eps", bufs=1)
            nc.tensor.matmul(
                out=rsu_ps[:], lhsT=xts[:, 0:1], rhs=pw_sb[:, 0, :],
                start=True, stop=False,
            )
            nc.tensor.matmul(
                out=rsu_ps[:], lhsT=xts[:, 1:2], rhs=pw_sb[:, 1, :],
                start=False, stop=True,
            )
            rsu_sb = ep.tile([1, FIN], F32, tag="rsu")
            nc.vector.tensor_tensor(
                out=rsu_sb[:], in0=rsu_ps[:], in1=pb_sb[:], op=mybir.AluOpType.add
            )
            rt_ps = pp.tile([128, 1], F32, tag="tp", bufs=1)
            nc.tensor.matmul(
                out=rt_ps[:], lhsT=rsu_sb[0:1, :], rhs=ones_sb[0:1, 0:1],
                start=True, stop=True,
            )
            rt_sb = ep.tile([128, 1], F32, tag="rt")
            nc.vector.tensor_copy(out=rt_sb[:], in_=rt_ps[:])
            sc_ps = pp.tile([1, FIN], F32, tag="eps", bufs=1)
            nc.tensor.matmul(
                out=sc_ps[:], lhsT=rt_sb[:], rhs=embt_sb[:], start=True, stop=True
            )
            fin_sb = ep.tile([1, 2 * FIN], F32, tag="fin")
            nc.vector.tensor_copy(out=fin_sb[0:1, 0:FIN], in_=sc_ps[:])
            nc.vector.tensor_copy(out=fin_sb[0:1, FIN : 2 * FIN], in_=rsu_sb[:])
            nc.sync.dma_start(out=out_p[:], in_=fin_sb[:])

    nc.finalize()
    _split_excess_waits(nc)
    return nc


def kernel(node_feature, edge_index, items_ready_to_cache, gcn_w, gcn_b,
           bn_gamma, bn_beta, emb_table, proj_w, proj_b, _timing=None):
    node_feature = np.asarray(node_feature, dtype=np.float32)
    edge_index = np.asarray(edge_index)
    gcn_w = np.asarray(gcn_w, dtype=np.float32)
    bn_gamma = np.asarray(bn_gamma, dtype=np.float32)
    bn_beta = np.asarray(bn_beta, dtype=np.float32)
    emb_table = np.asarray(emb_table, dtype=np.float32)
    proj_w = np.asarray(proj_w, dtype=np.float32)
    proj_b = np.asarray(proj_b, dtype=np.float32)

    dinv, sched, nsub, ncalls, per_core = _prep_host(edge_index)
    tot_sub = sum(nsub)
    tot_call = sum(ncalls)
    print(f"[kernel] subgroups/core={tot_sub} gathered={tot_sub*128} "
          f"real={(np.asarray(edge_index).shape[1])/NCORE:.0f} calls={tot_call}")

    nc = _build_graph(sched, nsub, ncalls)

    xs_scaled = node_feature * dinv[:, None]
    xt_full = np.zeros((128, NPAD), dtype=ml_dtypes.bfloat16)
    xt_full[:, :N] = xs_scaled.T.astype(ml_dtypes.bfloat16)
    w_bf = gcn_w.astype(ml_dtypes.bfloat16)
    dinv_pad = np.zeros(NPAD, dtype=np.float32)
    dinv_pad[:N] = dinv
    dinv_arr = dinv_pad.reshape(NPAD // 128, 128).T.copy()
    iota_arr = np.tile(np.arange(128, dtype=np.float32)[None, None, :], (128, SUPER, 1))
    embt = emb_table.T.copy()
    common = {
        "xt": xt_full,
        "w": w_bf,
        "dinv": dinv_arr,
        "iota": iota_arr,
        "embt": embt,
        "pw": proj_w,
        "gamma": bn_gamma[None, :],
        "beta": bn_beta[None, :],
        "pb": proj_b[None, :],
    }
    in_maps = []
    for c in range(NCORE):
        dd = np.zeros((128, NT), dtype=np.float32)
        dv = dinv[c * NL : (c + 1) * NL]
        dvp = np.zeros(NT * 128, dtype=np.float32)
        dvp[:NL] = dv
        dd[:, :] = dvp.reshape(NT, 128).T
        xo = np.zeros((128, NT * 128), dtype=ml_dtypes.bfloat16)
        xo[:, :NL] = node_feature.T[:, c * NL : (c + 1) * NL].astype(ml_dtypes.bfloat16)
        m = dict(common)
        m["dinvd"] = dd
        m["xown"] = xo
        m.update(per_core[c])
        in_maps.append(m)

    kw = {}
    if _timing is not None:
        kw = dict(trace=True, tmpdir=_timing.get("tmpdir"))
    res = run_bass_kernel_spmd(nc, in_maps, core_ids=list(range(NCORE)), **kw)
    if _timing is not None:
        _timing["exec_time_ns"] = res.exec_time_ns
        _timing["profile_json"] = res.profile_json
    out0 = np.asarray(res.results[0]["out"], dtype=np.float32).reshape(2, FIN)
    scores = out0[0]
    rsu = out0[1:2]
    return scores, rsu


if __name__ == "__main__":
    rng = np.random.default_rng(0)
    ei = rng.integers(0, N, (2, E)).astype(np.int64)
    x = rng.standard_normal((N, FIN)).astype(np.float32)
    out = kernel(
        node_feature=x,
        edge_index=ei,
        items_ready_to_cache=np.arange(128),
        gcn_w=rng.standard_normal((FIN, FH)).astype(np.float32) / math.sqrt(FIN),
        gcn_b=np.zeros(FH, np.float32),
        bn_gamma=np.ones(FH, np.float32),
        bn_beta=np.zeros(FH, np.float32),
        emb_table=rng.standard_normal((FIN, FIN)).astype(np.float32),
        proj_w=rng.standard_normal((FH, FIN)).astype(np.float32) / math.sqrt(FH),
        proj_b=np.zeros(FIN, np.float32),
    )
    print("scores", out[0][:4], "rsu", out[1][0, :4])


# revision 21
# speedup vs baseline: 1.5421x; 1.2179x over previous
"""GCN message-passing kernel for 8 TRN2 NeuronCores.

Math (reference): h = X @ W;  out[d] = sum_{e:(s->d)} dinv[s]*dinv[d]*h[s]
(+ self loop);  BN(train) over nodes; relu; row0 -> proj -> scores.

Device strategy (all 8 cores SPMD, node-sharded destinations):
 - dinv[s] is folded into h at phase 1 (h' = dinv[v] * (X@W)[v], bf16 rows
   in DRAM); dinv[d] is applied after aggregation. BN bias cancels.
 - Each core aggregates only its 12500 dst nodes: host sorts its in-edges
   by (dst_tile, src_chunk of 32768), pads runs to 128-edge subgroups.
 - dma_gather pulls h' rows (512B) into edge-per-partition tiles; 0/1
   selection matrices built on-device (is_equal vs iota) turn scatter-add
   into PSUM matmul accumulation. SBUF f32 accumulator per dst tile.
 - BN stats via ones-vector matmuls, 2KB AllReduce, epilogue on device.
"""
import math
import numpy as np
import ml_dtypes

from concourse import bass, bacc, mybir, tile
from concourse.bass_utils import run_bass_kernel_spmd
from concourse.masks import make_identity

F32 = mybir.dt.float32
BF16 = mybir.dt.bfloat16
I16 = mybir.dt.int16

N = 100000
E = 1600000
FIN = 128
FH = 256
NCORE = 8
NL = N // NCORE          # 12500 dst nodes per core
NT = (NL + 127) // 128   # 98 dst tiles per core
NPAD = 100352            # 196 * 512 padded nodes for phase 1
CH = 32768               # src chunk (int16 gather index range)
NCHUNK = 4
CH_ROWS = [CH, CH, CH, NPAD - 3 * CH]  # [32768,32768,32768,2048]
GCALL = 2048             # indices per dma_gather call
SUPER = 8                # subgroups per S-build op
BN_EPS = 1e-5


def _split_excess_waits(nc, maxw=1):
    """This walrus build supports only one sync wait per instruction; move
    extras onto preceding same-engine drains."""
    for blk in nc.main_func.blocks:
        insts = list(blk.instructions)
        newlist, changed = [], False
        for inst in insts:
            si = getattr(inst, "sync_info", None)
            if si is not None and si.on_wait is not None and len(si.on_wait) > maxw:
                waits = list(si.on_wait)
                pre, keep = waits[:-maxw], waits[-maxw:]
                for i in range(0, len(pre), maxw):
                    d = mybir.InstNoOp(name=f"{inst.name}-ws{i}", ins=[], outs=[])
                    d.engine = inst.engine
                    d.sync_info = mybir.SyncInfo(on_wait=pre[i : i + maxw], on_update=[])
                    newlist.append(d)
                si.on_wait = keep
                changed = True
            newlist.append(inst)
        if changed:
            blk.instructions = newlist


def _wrap_calls(idx, ncalls):
    """Per-GCALL-window wrapped int16 index layout [128, ncalls*128]."""
    cols = []
    for g in range(ncalls):
        w = idx[g * GCALL : (g + 1) * GCALL].reshape(GCALL // 16, 16).T  # [16,128]
        cols.append(np.tile(w, (8, 1)))
    return np.concatenate(cols, axis=1).astype(np.int16)


def _prep_host(edge_index):
    """Degree/norm plus per-core padded edge streams. Returns (dinv, sched,
    per_core_arrays) where sched[ch] = [(t, r), ...] shared by all cores."""
    src = np.asarray(edge_index[0], dtype=np.int64)
    dst = np.asarray(edge_index[1], dtype=np.int64)
    deg = np.bincount(dst, minlength=N).astype(np.float64) + 1.0
    dinv = (1.0 / np.sqrt(deg)).astype(np.float32)

    s_all = src
    d_all = dst

    core_edges = []  # (s_local_in_chunk(int32), tile, chunk) sorted per core
    counts = np.zeros((NCORE, NT, NCHUNK), dtype=np.int64)
    for c in range(NCORE):
        sel = (d_all >= c * NL) & (d_all < (c + 1) * NL)
        s_c = s_all[sel]
        d_c = d_all[sel] - c * NL
        t_id = d_c >> 7
        ch_id = s_c >> 15
        order = np.lexsort((s_c, ch_id, t_id))
        s_c, d_c, t_id, ch_id = s_c[order], d_c[order], t_id[order], ch_id[order]
        core_edges.append((s_c, d_c, t_id, ch_id))
        np.add.at(counts[c], (t_id, ch_id), 1)

    # Unpadded pair schedule: per chunk, each core's edge stream is the
    # concatenation of its (tile-sorted) edges, padded only at the end to a
    # common per-call multiple. Subgroups (128-edge windows) may straddle
    # tile boundaries; each (tile, subgroup) PAIR gets its own selection
    # matrix whose out-of-tile rows are zero. The pair schedule is the
    # union over cores so the graph is SPMD-uniform.
    cnt_tc = counts  # [NCORE, NT, NCHUNK]
    L_ch = cnt_tc.sum(axis=1).max(axis=0)  # [NCHUNK] max edges per chunk
    ncalls = [int(-(-int(L_ch[ch]) // GCALL)) for ch in range(NCHUNK)]
    nsub = [ncalls[ch] * GCALL // 128 for ch in range(NCHUNK)]

    sched = []   # sched[ch] = (pair_list, tile_ranges) ; pair_list = [(t, s), ...]
    for ch in range(NCHUNK):
        cum = np.zeros((NCORE, NT + 1), dtype=np.int64)
        cum[:, 1:] = np.cumsum(cnt_tc[:, :, ch], axis=1)
        tile_ranges = []
        for t in range(NT):
            s0 = int(cum[:, t].min() // 128)
            e_max = int(cum[:, t + 1].max())
            s1 = int(-(-e_max // 128)) if e_max > cum[:, t].min() else s0
            s1 = max(s1, s0)
            tile_ranges.append((s0, s1))
        pair_list = [(t, sg) for t in range(NT) for sg in range(*tile_ranges[t])]
        sched.append((pair_list, tile_ranges))
        if ch in (0, 2):
            assert all(r[1] > r[0] for r in tile_ranges), f"empty tile run in chunk {ch}"

    per_core = []
    for c in range(NCORE):
        s_c, d_c, t_id, ch_id = core_edges[c]
        arrays = {}
        for ch in range(NCHUNK):
            sel = ch_id == ch
            sc_ch = s_c[sel] - ch * CH
            dc_ch = d_c[sel]
            tc_ch = t_id[sel]
            L = ncalls[ch] * GCALL
            n_real = len(sc_ch)
            idx_stream = np.zeros(L, dtype=np.int64)
            idx_stream[:n_real] = sc_ch
            tile_stream = np.full(L, -1, dtype=np.int64)
            tile_stream[:n_real] = tc_ch
            rel_stream = np.zeros(L, dtype=np.float32)
            rel_stream[:n_real] = (dc_ch - tc_ch * 128).astype(np.float32)
            arrays[f"gidx{ch}"] = _wrap_calls(idx_stream, ncalls[ch])
            pair_list, _ = sched[ch]
            meta = np.full((128, len(pair_list)), -1.0, dtype=np.float32)
            tile_mat = tile_stream.reshape(nsub[ch], 128).T  # [128, nsub]
            rel_mat = rel_stream.reshape(nsub[ch], 128).T
            for k, (t, sg) in enumerate(pair_list):
                col = np.where(tile_mat[:, sg] == t, rel_mat[:, sg], -1.0)
                meta[:, k] = col
            arrays[f"meta{ch}"] = meta
        per_core.append(arrays)

    nmeta = [len(sched[ch][0]) for ch in range(NCHUNK)]
    return dinv, sched, nsub, ncalls, nmeta, per_core


def _build_graph(sched, nsub, ncalls, nmeta):
    nc = bacc.Bacc(None)
    xt = nc.declare_dram_parameter("xt", [128, NPAD], BF16, isOutput=False)
    xown_p = nc.declare_dram_parameter("xown", [128, NT * 128], BF16, isOutput=False)
    w = nc.declare_dram_parameter("w", [128, FH], BF16, isOutput=False)
    dinv_p = nc.declare_dram_parameter("dinv", [128, NPAD // 128], F32, isOutput=False)
    dinvd_p = nc.declare_dram_parameter("dinvd", [128, NT], F32, isOutput=False)
    iota_p = nc.declare_dram_parameter("iota", [128, SUPER, 128], F32, isOutput=False)
    embt_p = nc.declare_dram_parameter("embt", [128, 128], F32, isOutput=False)
    pw_p = nc.declare_dram_parameter("pw", [FH, FIN], F32, isOutput=False)
    gamma_p = nc.declare_dram_parameter("gamma", [1, FH], F32, isOutput=False)
    beta_p = nc.declare_dram_parameter("beta", [1, FH], F32, isOutput=False)
    pb_p = nc.declare_dram_parameter("pb", [1, FIN], F32, isOutput=False)
    gidx_p = [
        nc.declare_dram_parameter(f"gidx{ch}", [128, ncalls[ch] * 128], I16, isOutput=False)
        for ch in range(NCHUNK)
    ]
    meta_p = [
        nc.declare_dram_parameter(f"meta{ch}", [128, nmeta[ch]], F32, isOutput=False)
        for ch in range(NCHUNK)
    ]
    out_p = nc.declare_dram_parameter("out", [1, 2 * FIN], F32, isOutput=True)

    with tile.TileContext(nc) as tc:
        with (
            tc.tile_pool(name="consts", bufs=1) as cp,
            tc.tile_pool(name="acc", bufs=1) as accp,
            tc.tile_pool(name="work", bufs=2) as wp,
            tc.tile_pool(name="psum", bufs=1, space="PSUM") as pp,
            tc.tile_pool(name="dram", bufs=1, space="DRAM") as dp,
        ):
            # ---- constants into SBUF ----
            w_sb = cp.tile([128, FH], BF16)
            nc.sync.dma_start(out=w_sb[:], in_=w[:])
            dinvd_sb = cp.tile([128, NT], F32)
            nc.sync.dma_start(out=dinvd_sb[:], in_=dinvd_p[:])
            iota_sb = cp.tile([128, SUPER, 128], F32)
            nc.sync.dma_start(out=iota_sb[:], in_=iota_p[:])
            embt_sb = cp.tile([128, 128], F32)
            nc.sync.dma_start(out=embt_sb[:], in_=embt_p[:])
            pw_sb = cp.tile([128, 2, FIN], F32)
            nc.sync.dma_start(
                out=pw_sb[:], in_=pw_p[:].rearrange("(j p) f -> p j f", p=128)
            )
            gamma_sb = cp.tile([1, FH], F32)
            nc.sync.dma_start(out=gamma_sb[:], in_=gamma_p[:])
            beta_sb = cp.tile([1, FH], F32)
            nc.sync.dma_start(out=beta_sb[:], in_=beta_p[:])
            pb_sb = cp.tile([1, FIN], F32)
            nc.sync.dma_start(out=pb_sb[:], in_=pb_p[:])
            ones_sb = cp.tile([128, 1], F32)
            nc.gpsimd.memset(ones_sb[:], 1.0)
            gidx_sb = [None] * NCHUNK
            meta_sb = [None] * NCHUNK
            def load_chunk_meta(ch):
                g = cp.tile([128, ncalls[ch] * 128], I16, tag=f"gidx{ch}", name=f"gidx{ch}")
                nc.sync.dma_start(out=g[:], in_=gidx_p[ch][:])
                gidx_sb[ch] = g
                m = cp.tile([128, nmeta[ch]], F32, tag=f"meta{ch}", name=f"meta{ch}")
                nc.sync.dma_start(out=m[:], in_=meta_p[ch][:])
                meta_sb[ch] = m
            load_chunk_meta(3)
            load_chunk_meta(0)

            h_dram = [
                dp.tile([CH_ROWS[ch], FH], BF16, tag=f"h{ch}", name=f"h{ch}")
                for ch in range(NCHUNK)
            ]

            # ---- phase 1: h' = dinv * (X @ W), bf16, to DRAM ----
            b5_order = list(range(192, NPAD // 512)) + list(range(192))
            for b5 in b5_order:
                o = b5 * 512
                xt_t = wp.tile([128, 512], BF16, tag="xt", bufs=3)
                nc.sync.dma_start(out=xt_t[:], in_=xt[:, o : o + 512])
                hs = wp.tile([128, 4, FH], BF16, tag="hs", bufs=3)
                for j2 in range(2):
                    hp = pp.tile([128, 2, FH], F32, tag="hpw", bufs=2)
                    for jj in range(2):
                        j = j2 * 2 + jj
                        nc.tensor.matmul(
                            out=hp[:, jj, :],
                            lhsT=xt_t[:, j * 128 : (j + 1) * 128],
                            rhs=w_sb[:],
                            start=True,
                            stop=True,
                        )
                    if j2 == 0:
                        nc.scalar.activation(
                            out=hs[:, 0:2, :],
                            in_=hp[:],
                            func=mybir.ActivationFunctionType.Copy,
                        )
                    else:
                        nc.vector.tensor_copy(out=hs[:, 2:4, :], in_=hp[:])
                ch = o // CH
                ro = o - ch * CH
                nc.sync.dma_start(
                    out=h_dram[ch][ro : ro + 512, :].rearrange("(j p) f -> p j f", p=128),
                    in_=hs[:],
                )

            # ---- phase 2: aggregation (stats for tile t fire after its last chunk) ----
            xacc = [accp.tile([128, FH], F32, tag=f"xacc{t}", name=f"xacc{t}") for t in range(NT)]
            sum_ps = pp.tile([1, FH], F32, tag="stsum")
            sq_ps = pp.tile([1, FH], F32, tag="stsq")
            x0_sb = cp.tile([1, FH], F32)

            def emit_stats(t):
                xo_t = wp.tile([128, 128], BF16, tag="xo", bufs=3)
                nc.sync.dma_start(
                    out=xo_t[:], in_=xown_p[:, t * 128 : (t + 1) * 128]
                )
                hop = pp.tile([128, FH], F32, tag="acc", bufs=2)
                nc.tensor.matmul(
                    out=hop[:], lhsT=xo_t[:], rhs=w_sb[:], start=True, stop=True
                )
                hos = wp.tile([128, FH], F32, tag="hos", bufs=2)
                nc.scalar.activation(
                    out=hos[:],
                    in_=hop[:],
                    func=mybir.ActivationFunctionType.Copy,
                    scale=dinvd_sb[:, t : t + 1],
                )
                nc.vector.tensor_tensor(
                    out=hos[:], in0=hos[:], in1=xacc[t][:], op=mybir.AluOpType.add
                )
                xs = wp.tile([128, FH], F32, tag="xs", bufs=3)
                nc.scalar.activation(
                    out=xs[:],
                    in_=hos[:],
                    func=mybir.ActivationFunctionType.Copy,
                    scale=dinvd_sb[:, t : t + 1],
                )
                sq = wp.tile([128, FH], F32, tag="sq", bufs=3)
                nc.scalar.square(out=sq[:], in_=xs[:])
                nc.tensor.matmul(
                    out=sum_ps[:], lhsT=ones_sb[:], rhs=xs[:],
                    start=(t == 0), stop=(t == NT - 1), skip_group_check=True,
                )
                nc.tensor.matmul(
                    out=sq_ps[:], lhsT=ones_sb[:], rhs=sq[:],
                    start=(t == 0), stop=(t == NT - 1), skip_group_check=True,
                )
                if t == 0:
                    nc.vector.tensor_copy(out=x0_sb[:], in_=xs[0:1, :])
            load_chunk_meta(1)
            load_chunk_meta(2)
            init_done = [False] * NT
            for ch in [3, 0, 1, 2]:
                pair_list, tile_ranges = sched[ch]
                gath = {}
                scur = {}

                def get_gather(g, ch=ch):
                    if g not in gath:
                        for old_g in [k for k in gath if k < g - 1]:
                            del gath[old_g]
                        gt = wp.tile([128, 16, FH], BF16, tag="gath", bufs=3)
                        nc.gpsimd.dma_gather(
                            out_ap=gt[:],
                            in_ap=h_dram[ch][:],
                            idxs_ap=gidx_sb[ch][:, g * 128 : (g + 1) * 128],
                            num_idxs=GCALL,
                            num_idxs_reg=GCALL,
                            elem_size=FH,
                            single_packet=False,
                        )
                        gath[g] = gt
                    return gath[g]

                def get_s(k, ch=ch):
                    if k not in scur:
                        k0 = k - (k % SUPER)
                        nsg = min(SUPER, nmeta[ch] - k0)
                        st = wp.tile([128, SUPER, 128], BF16, tag="sbuild", bufs=2)
                        nc.vector.tensor_tensor(
                            out=st[:, :nsg, :],
                            in0=meta_sb[ch][:, k0 : k0 + nsg].to_broadcast(
                                [128, nsg, 128]
                            ),
                            in1=iota_sb[:, :nsg, :],
                            op=mybir.AluOpType.is_equal,
                        )
                        scur.clear()
                        scur.update({k0 + i: (st, i) for i in range(nsg)})
                    return scur[k]

                k = 0
                for t in range(NT):
                    s0, s1 = tile_ranges[t]
                    if s1 <= s0:
                        if ch == 2:
                            emit_stats(t)
                        continue
                    ps = pp.tile([128, FH], F32, tag="acc", bufs=2)
                    for sg in range(s0, s1):
                        gt = get_gather(sg // 16)
                        st, i = get_s(k)
                        nc.tensor.matmul(
                            out=ps[:],
                            lhsT=st[:, i, :],
                            rhs=gt[:, sg % 16, :],
                            start=(sg == s0),
                            stop=(sg == s1 - 1),
                        )
                        k += 1
                    if not init_done[t]:
                        nc.vector.tensor_copy(out=xacc[t][:], in_=ps[:])
                        init_done[t] = True
                    else:
                        nc.vector.tensor_tensor(
                            out=xacc[t][:],
                            in0=xacc[t][:],
                            in1=ps[:],
                            op=mybir.AluOpType.add,
                        )
                    if ch == 2:
                        emit_stats(t)
                assert k == nmeta[ch]

            # ---- AllReduce the stats ----
            st_sb = cp.tile([1, 2 * FH], F32)
            nc.vector.tensor_copy(out=st_sb[0:1, 0:FH], in_=sum_ps[:])
            nc.vector.tensor_copy(out=st_sb[0:1, FH : 2 * FH], in_=sq_ps[:])
            arin = dp.tile([1, 2 * FH], F32, tag="arin")
            arout = dp.tile([1, 2 * FH], F32, tag="arout")
            nc.sync.dma_start(out=arin[:], in_=st_sb[:])
            nc.gpsimd.collective_compute(
                "AllReduce",
                mybir.AluOpType.add,
                replica_groups=[list(range(NCORE))],
                ins=[arin.opt()],
                outs=[arout.opt()],
            )
            st2_sb = cp.tile([1, 2 * FH], F32)
            nc.sync.dma_start(out=st2_sb[:], in_=arout[:])

            # ---- epilogue (all cores compute; only core0's out is used) ----
            ep = cp
            m = ep.tile([1, FH], F32, tag="m")
            nc.scalar.activation(
                out=m[:], in_=st2_sb[0:1, 0:FH],
                func=mybir.ActivationFunctionType.Copy, scale=1.0 / N,
            )
            e2 = ep.tile([1, FH], F32, tag="e2")
            nc.scalar.activation(
                out=e2[:], in_=st2_sb[0:1, FH : 2 * FH],
                func=mybir.ActivationFunctionType.Copy, scale=1.0 / N,
            )
            msq = ep.tile([1, FH], F32, tag="msq")
            nc.scalar.square(out=msq[:], in_=m[:])
            var = ep.tile([1, FH], F32, tag="var")
            nc.vector.tensor_tensor(
                out=var[:], in0=e2[:], in1=msq[:], op=mybir.AluOpType.subtract
            )
            nc.vector.tensor_scalar_add(var[:], var[:], BN_EPS)
            std = ep.tile([1, FH], F32, tag="std")
            nc.scalar.sqrt(out=std[:], in_=var[:])
            rstd = ep.tile([1, FH], F32, tag="rstd")
            nc.vector.reciprocal(out=rstd[:], in_=std[:])
            xc = ep.tile([1, FH], F32, tag="xc")
            nc.vector.tensor_tensor(
                out=xc[:], in0=x0_sb[:], in1=m[:], op=mybir.AluOpType.subtract
            )
            nc.vector.tensor_tensor(
                out=xc[:], in0=xc[:], in1=rstd[:], op=mybir.AluOpType.mult
            )
            nc.vector.tensor_tensor(
                out=xc[:], in0=xc[:], in1=gamma_sb[:], op=mybir.AluOpType.mult
            )
            nc.vector.tensor_tensor(
                out=xc[:], in0=xc[:], in1=beta_sb[:], op=mybir.AluOpType.add
            )
            xr = ep.tile([1, FH], F32, tag="xr")
            nc.scalar.activation(
                out=xr[:], in_=xc[:], func=mybir.ActivationFunctionType.Relu
            )
            # transpose xr -> [256,1] via K=1 matmuls with ones
            xts = ep.tile([128, 2], F32, tag="xts")
            for half in range(2):
                tp = pp.tile([128, 1], F32, tag="tp", bufs=1)
                nc.tensor.matmul(
                    out=tp[:],
                    lhsT=xr[0:1, half * 128 : (half + 1) * 128],
                    rhs=ones_sb[0:1, 0:1],
                    start=True,
                    stop=True,
                )
                nc.vector.tensor_copy(out=xts[:, half : half + 1], in_=tp[:])
            rsu_ps = pp.tile([1, FIN], F32, tag="eps", bufs=1)
            nc.tensor.matmul(
                out=rsu_ps[:], lhsT=xts[:, 0:1], rhs=pw_sb[:, 0, :],
                start=True, stop=False,
            )
            nc.tensor.matmul(
                out=rsu_ps[:], lhsT=xts[:, 1:2], rhs=pw_sb[:, 1, :],
                start=False, stop=True,
            )
            rsu_sb = ep.tile([1, FIN], F32, tag="rsu")
            nc.vector.tensor_tensor(
                out=rsu_sb[:], in0=rsu_ps[:], in1=pb_sb[:], op=mybir.AluOpType.add
            )
            rt_ps = pp.tile([128, 1], F32, tag="tp", bufs=1)
            nc.tensor.matmul(
                out=rt_ps[:], lhsT=rsu_sb[0:1, :], rhs=ones_sb[0:1, 0:1],
                start=True, stop=True,
            )
            rt_sb = ep.tile([128, 1], F32, tag="rt")
            nc.vector.tensor_copy(out=rt_sb[:], in_=rt_ps[:])
            sc_ps = pp.tile([1, FIN], F32, tag="eps", bufs=1)
            nc.tensor.matmul(
                out=sc_ps[:], lhsT=rt_sb[:], rhs=embt_sb[:], start=True, stop=True
            )
            fin_sb = ep.tile([1, 2 * FIN], F32, tag="fin")
            nc.vector.tensor_copy(out=fin_sb[0:1, 0:FIN], in_=sc_ps[:])
            nc.vector.tensor_copy(out=fin_sb[0:1, FIN : 2 * FIN], in_=rsu_sb[:])
            nc.sync.dma_start(out=out_p[:], in_=fin_sb[:])

    nc.finalize()
    _split_excess_waits(nc)
    return nc


def kernel(node_feature, edge_index, items_ready_to_cache, gcn_w, gcn_b,
           bn_gamma, bn_beta, emb_table, proj_w, proj_b, _timing=None):
    node_feature = np.asarray(node_feature, dtype=np.float32)
    edge_index = np.asarray(edge_index)
    gcn_w = np.asarray(gcn_w, dtype=np.float32)
    bn_gamma = np.asarray(bn_gamma, dtype=np.float32)
    bn_beta = np.asarray(bn_beta, dtype=np.float32)
    emb_table = np.asarray(emb_table, dtype=np.float32)
    proj_w = np.asarray(proj_w, dtype=np.float32)
    proj_b = np.asarray(proj_b, dtype=np.float32)

    dinv, sched, nsub, ncalls, nmeta, per_core = _prep_host(edge_index)
    print(f"[kernel] gathered={sum(nsub)*128} real={np.asarray(edge_index).shape[1]/NCORE:.0f} "
          f"calls={sum(ncalls)} pairs={sum(nmeta)}")

    nc = _build_graph(sched, nsub, ncalls, nmeta)

    xs_scaled = node_feature * dinv[:, None]
    xt_full = np.zeros((128, NPAD), dtype=ml_dtypes.bfloat16)
    xt_full[:, :N] = xs_scaled.T.astype(ml_dtypes.bfloat16)
    w_bf = gcn_w.astype(ml_dtypes.bfloat16)
    dinv_pad = np.zeros(NPAD, dtype=np.float32)
    dinv_pad[:N] = dinv
    dinv_arr = dinv_pad.reshape(NPAD // 128, 128).T.copy()
    iota_arr = np.tile(np.arange(128, dtype=np.float32)[None, None, :], (128, SUPER, 1))
    embt = emb_table.T.copy()
    common = {
        "xt": xt_full,
        "w": w_bf,
        "dinv": dinv_arr,
        "iota": iota_arr,
        "embt": embt,
        "pw": proj_w,
        "gamma": bn_gamma[None, :],
        "beta": bn_beta[None, :],
        "pb": proj_b[None, :],
    }
    in_maps = []
    for c in range(NCORE):
        dd = np.zeros((128, NT), dtype=np.float32)
        dv = dinv[c * NL : (c + 1) * NL]
        dvp = np.zeros(NT * 128, dtype=np.float32)
        dvp[:NL] = dv
        dd[:, :] = dvp.reshape(NT, 128).T
        xo = np.zeros((128, NT * 128), dtype=ml_dtypes.bfloat16)
        xo[:, :NL] = node_feature.T[:, c * NL : (c + 1) * NL].astype(ml_dtypes.bfloat16)
        m = dict(common)
        m["dinvd"] = dd
        m["xown"] = xo
        m.update(per_core[c])
        in_maps.append(m)

    kw = {}
    if _timing is not None:
        kw = dict(trace=True, tmpdir=_timing.get("tmpdir"))
    res = run_bass_kernel_spmd(nc, in_maps, core_ids=list(range(NCORE)), **kw)
    if _timing is not None:
        _timing["exec_time_ns"] = res.exec_time_ns
        _timing["profile_json"] = res.profile_json
    out0 = np.asarray(res.results[0]["out"], dtype=np.float32).reshape(2, FIN)
    scores = out0[0]
    rsu = out0[1:2]
    return scores, rsu


if __name__ == "__main__":
    rng = np.random.default_rng(0)
    ei = rng.integers(0, N, (2, E)).astype(np.int64)
    x = rng.standard_normal((N, FIN)).astype(np.float32)
    out = kernel(
        node_feature=x,
        edge_index=ei,
        items_ready_to_cache=np.arange(128),
        gcn_w=rng.standard_normal((FIN, FH)).astype(np.float32) / math.sqrt(FIN),
        gcn_b=np.zeros(FH, np.float32),
        bn_gamma=np.ones(FH, np.float32),
        bn_beta=np.zeros(FH, np.float32),
        emb_table=rng.standard_normal((FIN, FIN)).astype(np.float32),
        proj_w=rng.standard_normal((FH, FIN)).astype(np.float32) / math.sqrt(FH),
        proj_b=np.zeros(FIN, np.float32),
    )
    print("scores", out[0][:4], "rsu", out[1][0, :4])
